# revision 36
# baseline (speedup 1.0000x reference)
"""LIAFResBlock forward on 8 Trainium2 NeuronCores (data-parallel over batch).

Self-contained: hardcodes shapes for x [16,64,8,56,56] -> out [16,128,8,28,28].

Math notes (vs the PyTorch/JAX reference):
  - conv biases are no-ops: every conv is followed by training-mode BN, which
    subtracts the per-channel mean, absorbing any per-channel constant.
  - the final mem_update on a binary {0,1} tensor is the identity because
    d = sigmoid(0.5) ~ 0.6225 and d*0.5 < 0.5, so out = lif_act(bn2(cv2)+bn_sc(sc)).
  - the first mem_update runs in "normalized" space: with a1 = g1*rstd1 (>0),
    v = m/a1 satisfies v[t] = d*v[t-1]*[v<=tau] + (cv1[t] + beta1/a1),
    spike[t] = v[t] > tau, tau = 0.5/a1. BN1 folds into a per-channel bias on
    cv1 (applied per-tile on the Act engine) plus a per-channel threshold.
  - BN batch stats are global over B=16: each core computes per-channel
    (sum, sumsq) partials; a tiny AllReduce combines them.

Performance notes:
  - conv1/shortcut run as bf16 hi/lo splits (2 matmuls per tap at 1 PE
    cycle/row each, ~2^-18 error vs fp32's 4 cycles/row); conv2 consumes
    exact binary spikes and runs single-pass float32r (1 cycle/row at
    free-size>=256; hardware f32r keeps ~13 mantissa bits).
  - the host pre-pads x to 57x57 with a leading zero row+column, so the
    x DMAs are fully contiguous (6.5KB/partition descriptors) and conv1's
    zero padding is free in-layout; all 9 taps are full matmuls.
  - weights are pre-transposed/split on the host so weight DMAs are
    contiguous.
  - output is written as uint8 spikes, one batched DMA per sample; the host
    casts back to fp32.
"""
import math
import sys

import numpy as np

sys.path.insert(0, "/opt/trn_rl_repo")

import concourse.bass as bass  # noqa: E402
import concourse.bacc as bacc  # noqa: E402
import concourse.tile as tile  # noqa: E402
from concourse import mybir  # noqa: E402
from concourse.bass_utils import run_bass_kernel_spmd  # noqa: E402

dt = mybir.dt
Alu = mybir.AluOpType
Act = mybir.ActivationFunctionType

B, CIN, COUT, T, H, W = 16, 64, 128, 8, 56, 56
WP = W + 1              # host-padded width: one leading zero column
HP = H + 1              # host-padded height: one leading zero row
HO = WO = 28
NPIX = HO * WO          # 784
RC = 14                 # output rows per PSUM chunk
CHUNK = RC * WO         # 392 (one PSUM bank)
NCORES = 8
BPC = B // NCORES       # 2 samples per core
NT = BPC * T            # 16 (s,t) tiles per core
NLOC = BPC * T * NPIX   # 12544 elements/channel per core
NGLOB = B * T * NPIX    # 100352 elements/channel globally
EPS = 1e-5
HS, WS = HO + 2, WO + 2  # 30x30 padded spike tile for conv2

f32r = dt.float32r


def _ap(base, off, free):
    """Sub-view of a 2D/3D SBUF AP: keep partition dim, custom free dims."""
    return bass.AP(tensor=base.tensor, offset=base.offset + off,
                   ap=[base.ap[0]] + free)


def build_nc(d: float) -> bass.Bass:
    nc = bacc.Bacc("TRN2", target_bir_lowering=False, num_devices=NCORES)

    # x and the conv weights are consumed exclusively by float32r matmuls;
    # declare them float32r end-to-end (same 32-bit container, the PE rounds
    # at ingest) so the BIR verifier sees consistent f32r producer chains
    # without any extra rounding copies.
    # conv1/shortcut run as bf16 hi/lo split (2 matmuls/tap, ~2^-18 error):
    #   y = [Whi;Wlo]@[Xhi;Xhi] (K=128) + Whi@Xlo (K=64)   (drops Wlo*Xlo)
    # xhh holds Xhi duplicated on both partition halves (host-prepared).
    xhh_d = nc.dram_tensor("xhh", [BPC, 2 * CIN, T, HP, WP], dt.bfloat16,
                           kind="ExternalInput")
    xlo_d = nc.dram_tensor("xlo", [BPC, CIN, T, HP, WP], dt.bfloat16,
                           kind="ExternalInput")
    # host pre-transposed: w1c[[hi;lo] i, k, o], w2t[i, k, o], wsc[[hi;lo] i, o]
    w1_d = nc.dram_tensor("w1c", [2 * CIN, 9, COUT], dt.bfloat16,
                          kind="ExternalInput")
    w2_d = nc.dram_tensor("w2t", [COUT, 9, COUT], f32r, kind="ExternalInput")
    ws_d = nc.dram_tensor("wsc", [2 * CIN, COUT], dt.bfloat16,
                          kind="ExternalInput")
    par_d = {}
    for p in ["bn1_g", "bn1_b", "bn2_g", "bn2_b", "scn_g", "scn_b"]:
        par_d[p] = nc.dram_tensor(p, [COUT, 1], dt.float32, kind="ExternalInput")
    out_d = nc.dram_tensor("out", [BPC, COUT, T, HO, WO], dt.uint8,
                           kind="ExternalOutput")

    from contextlib import ExitStack
    with tile.TileContext(nc) as tc, ExitStack() as stk:
        big = stk.enter_context(tc.tile_pool(name="big", bufs=1))
        const = stk.enter_context(tc.tile_pool(name="const", bufs=1))
        psum = stk.enter_context(tc.tile_pool(name="psum", bufs=6, space="PSUM"))
        dramp = stk.enter_context(tc.tile_pool(name="dramp", bufs=1, space="DRAM"))

        # ---- weights to SBUF (contiguous DMAs; w1/ws off the SP queue so
        # the first x-tile DMA leads; w2/params are not needed until after
        # phase A) ----
        w1 = const.tile([2 * CIN, 9, COUT], dt.bfloat16)
        nc.gpsimd.dma_start(out=w1[:, :, :], in_=w1_d[:, :, :])
        ws = const.tile([2 * CIN, COUT], dt.bfloat16)
        nc.gpsimd.dma_start(out=ws[:, :], in_=ws_d[:, :])

        params = {}
        for p, dten in par_d.items():
            tl = const.tile([COUT, 1], dt.float32, tag=p)
            nc.scalar.dma_start(out=tl[:, :], in_=dten[:, :])
            params[p] = tl
        eps_t = const.tile([COUT, 1], dt.float32)
        nc.vector.memset(eps_t[:, :], EPS)

        # ---- persistent activation buffers (per-channel partition layout) ----
        cv1f = big.tile([COUT, NLOC], dt.float32)   # conv1 raw
        scf = big.tile([COUT, NLOC], dt.float32)    # shortcut raw, later s'
        cv2f = big.tile([COUT, NLOC], dt.float32)   # conv2 raw
        st1 = const.tile([COUT, 2 * NT, 6], dt.float32)   # bn_stats conv1
        sts = const.tile([COUT, 2 * NT, 6], dt.float32)   # bn_stats shortcut
        st2 = const.tile([COUT, 2 * NT, 6], dt.float32)   # bn_stats conv2

        # ================= phase A: conv1 + shortcut =================
        # the host pads x with one leading zero row AND column, so tap
        # (kh,kw) of the stride-2 pad-1 3x3 conv reads padded element
        # (2r+kh, 2q+kw) -- always in range, all 9 taps are full matmuls.
        with tc.tile_pool(name="xt", bufs=4) as xpool:
            for s in range(BPC):
                for t in range(T):
                    it = s * T + t
                    xh = xpool.tile([2 * CIN, HP * WP], dt.bfloat16, tag="xh")
                    xl = xpool.tile([CIN, HP * WP], dt.bfloat16, tag="xl")
                    # spread x DMAs over two issue queues so their fixed
                    # costs pipeline instead of serializing on SP (not the
                    # Act queue: its exec-queue depth is 0, so PSUM-copy
                    # dispatches would delay the DMA issue)
                    dma_eng = (nc.sync, nc.gpsimd)[it % 2]
                    dma_eng.dma_start(
                        out=xh[:, :],
                        in_=xhh_d.ap()[s, :, t, :, :].rearrange("c h w -> c (h w)"))
                    dma_eng2 = (nc.sync, nc.gpsimd)[(it + 1) % 2]
                    dma_eng2.dma_start(
                        out=xl[:, :],
                        in_=xlo_d.ap()[s, :, t, :, :].rearrange("c h w -> c (h w)"))
                    xhb = xh[:, 0:1]
                    xlb = xl[:, 0:1]
                    for c in range(2):
                        ps1 = psum.tile([COUT, CHUNK], dt.float32, tag="mm")
                        for k in range(9):
                            kh, kw = divmod(k, 3)
                            base = (28 * c + kh) * WP + kw
                            free = [[2 * WP, RC], [2, WO]]
                            nc.tensor.matmul(ps1[:, :], w1[:, k, :],
                                             _ap(xhb, base, free),
                                             start=(k == 0), stop=False)
                            nc.tensor.matmul(ps1[:, :], w1[0:CIN, k, :],
                                             _ap(xlb, base, free),
                                             start=False, stop=(k == 8))
                        off = it * NPIX + c * CHUNK
                        nc.scalar.copy(cv1f[:, off:off + CHUNK], ps1[:, :])
                        nc.vector.bn_stats(out=st1[:, 2 * it + c, :], in_=ps1[:, :])
                        # shortcut 1x1 stride2: reads padded (2r+1, 2q+1)
                        ps2 = psum.tile([COUT, CHUNK], dt.float32, tag="mm")
                        base = (28 * c + 1) * WP + 1
                        free = [[2 * WP, RC], [2, WO]]
                        nc.tensor.matmul(ps2[:, :], ws[:, :],
                                         _ap(xhb, base, free),
                                         start=True, stop=False)
                        nc.tensor.matmul(ps2[:, :], ws[0:CIN, :],
                                         _ap(xlb, base, free),
                                         start=False, stop=True)
                        nc.scalar.copy(scf[:, off:off + CHUNK], ps2[:, :])
                        nc.vector.bn_stats(out=sts[:, 2 * it + c, :], in_=ps2[:, :])

        # w2 is first needed for conv2 (after AR1): load it into the space
        # the x-tile pool just freed, overlapping the AllGather window.
        midp = stk.enter_context(tc.tile_pool(name="midp", bufs=1))
        w2 = midp.tile([COUT, 9, COUT], f32r)
        nc.scalar.dma_start(out=w2[:, :, :], in_=w2_d[:, :, :])

        # ---- local stats -> (sum, sumsq) -> AllReduce #1 ----
        mv1 = const.tile([COUT, 2], dt.float32)
        nc.vector.bn_aggr(out=mv1[:, :], in_=st1[:, :, :])
        mvs = const.tile([COUT, 2], dt.float32)
        nc.vector.bn_aggr(out=mvs[:, :], in_=sts[:, :, :])
        ar1 = const.tile([COUT, 4], dt.float32)
        for mv, base in ((mv1, 0), (mvs, 2)):
            nc.vector.tensor_scalar_mul(ar1[:, base:base + 1], mv[:, 0:1],
                                        float(NLOC))
            # sumsq = (var + mean^2) * NLOC
            nc.vector.scalar_tensor_tensor(
                ar1[:, base + 1:base + 2], mv[:, 0:1], float(NLOC), mv[:, 0:1],
                Alu.mult, Alu.mult)
            nc.vector.scalar_tensor_tensor(
                ar1[:, base + 1:base + 2], mv[:, 1:2], float(NLOC),
                ar1[:, base + 1:base + 2], Alu.mult, Alu.add)
        # Cross-core stats sum. AllGather + local tree-reduce is ~13us
        # cheaper than AllReduce in the cost model, but its HW layout is
        # unverified; GATHER=False falls back to plain AllReduce.
        # AllGather + local reduce would be ~13us cheaper per sync in the
        # cost model, but the REAL hardware AllGather layout differs from
        # CoreSim's (verified broken on HW 2026-08-09) - keep AllReduce.
        GATHER = False

        def gathered_sum(src, width, tag):
            cci = dramp.tile([COUT, width], dt.float32, tag=tag + "_i")
            owid = NCORES * width if GATHER else width
            cco = dramp.tile([COUT, owid], dt.float32,
                             addr_space="Shared", tag=tag + "_o")
            nc.sync.dma_start(out=cci[:, :], in_=src)
            nc.gpsimd.collective_compute(
                "AllGather" if GATHER else "AllReduce",
                Alu.bypass if GATHER else Alu.add,
                replica_groups=[list(range(NCORES))],
                ins=[cci[:, :].opt()], outs=[cco[:, :].opt()])
            gg = const.tile([COUT, owid], dt.float32, tag=tag + "_g")
            nc.sync.dma_start(out=gg[:, :], in_=cco[:, :])
            if GATHER:
                for half in (4 * width, 2 * width, width):
                    nc.vector.tensor_tensor(gg[:, 0:half], gg[:, 0:half],
                                            gg[:, half:2 * half], Alu.add)
            return gg

        gs1 = gathered_sum(ar1[:, :], 4, "cc1")

        def mk_bn_consts(sums, g, b, tag):
            """global (sum,sumsq) [128,2] -> a = g*rstd, bb = b - a*mean."""
            mean = const.tile([COUT, 1], dt.float32, tag=tag + "_mean")
            nc.vector.tensor_scalar_mul(mean[:, :], sums[:, 0:1], 1.0 / NGLOB)
            var = const.tile([COUT, 1], dt.float32, tag=tag + "_var")
            nc.vector.tensor_scalar_mul(var[:, :], sums[:, 1:2], 1.0 / NGLOB)
            m2 = const.tile([COUT, 1], dt.float32, tag=tag + "_m2")
            nc.vector.tensor_tensor(m2[:, :], mean[:, :], mean[:, :], Alu.mult)
            nc.vector.tensor_tensor(var[:, :], var[:, :], m2[:, :], Alu.subtract)
            a = const.tile([COUT, 1], dt.float32, tag=tag + "_a")
            nc.scalar.activation(a[:, :], var[:, :], Act.Sqrt, bias=eps_t[:, :])
            nc.vector.reciprocal(a[:, :], a[:, :])
            nc.vector.tensor_tensor(a[:, :], a[:, :], g[:, :], Alu.mult)
            bb = const.tile([COUT, 1], dt.float32, tag=tag + "_bb")
            nc.vector.tensor_tensor(bb[:, :], a[:, :], mean[:, :], Alu.mult)
            nc.vector.tensor_tensor(bb[:, :], b[:, :], bb[:, :], Alu.subtract)
            return a, bb

        a1, b1 = mk_bn_consts(gs1[:, 0:2], params["bn1_g"], params["bn1_b"], "bn1")
        asc, bsc = mk_bn_consts(gs1[:, 2:4], params["scn_g"], params["scn_b"], "scn")

        # tau = 0.5/a1 ; beta~ = b1/a1  (a1 > 0 since gamma=1 at init)
        ra1 = const.tile([COUT, 1], dt.float32)
        nc.vector.reciprocal(ra1[:, :], a1[:, :])
        tau = const.tile([COUT, 1], dt.float32)
        nc.vector.tensor_scalar_mul(tau[:, :], ra1[:, :], 0.5)
        btil = const.tile([COUT, 1], dt.float32)
        nc.vector.tensor_tensor(btil[:, :], b1[:, :], ra1[:, :], Alu.mult)
        # shortcut transform consts: s' = -asc*sc + (0.5 - bsc), issued as a
        # bulk Pool op AFTER the phase-B loop (overlaps conv2 tail + AR2).
        nasc = const.tile([COUT, 1], dt.float32)
        nc.vector.tensor_scalar_mul(nasc[:, :], asc[:, :], -1.0)
        hbsc = const.tile([COUT, 1], dt.float32)
        nc.vector.tensor_scalar(hbsc[:, :], bsc[:, :], -1.0, 0.5, Alu.mult, Alu.add)

        # ================= phase B: LIF recurrence + conv2 =================
        with tc.tile_pool(name="cpp", bufs=4) as cpool, \
             tc.tile_pool(name="phu", bufs=3) as pu, \
             tc.tile_pool(name="phv", bufs=4) as pv, \
             tc.tile_pool(name="spk", bufs=3) as spool:
            v_prev = [None] * BPC
            for t in range(T):
                # hide the bulk shortcut transform s' = -asc*sc + (0.5-bsc)
                # on the Act engine mid-phase (it must precede the final
                # phase but nothing in phase B reads scf)
                if t in (4, 6):
                    half = NLOC // 2
                    lo = (t == 6) * half
                    nc.scalar.activation(scf[:, lo:lo + half], scf[:, lo:lo + half],
                                         Act.Identity, bias=hbsc[:, :],
                                         scale=nasc[:, :])
                for s in range(BPC):
                    it = s * T + t
                    off = it * NPIX
                    # c' = cv1 + beta~ (per-tile, Act engine)
                    cp = cpool.tile([COUT, NPIX], dt.float32, tag="cp")
                    nc.scalar.activation(cp[:, :], cv1f[:, off:off + NPIX],
                                         Act.Identity, bias=btil[:, :])
                    if t == 0:
                        v = cp[:, :]
                    else:
                        u = pu.tile([COUT, NPIX], dt.float32, tag="u")
                        nc.vector.scalar_tensor_tensor(
                            u[:, :], v_prev[s], tau[:, :], v_prev[s],
                            Alu.is_le, Alu.mult)
                        vt = pv.tile([COUT, NPIX], dt.float32, tag="v")
                        nc.vector.scalar_tensor_tensor(
                            vt[:, :], u[:, :], float(d), cp[:, :],
                            Alu.mult, Alu.add)
                        v = vt[:, :]
                    v_prev[s] = v
                    # spikes -> zero-ringed padded tile ({0,1} exact in f32r)
                    sp = spool.tile([COUT, HS, WS], f32r, tag="sp")
                    rings = [sp[:, 0, :], sp[:, HS - 1, :],
                             _ap(sp[:, 0, 0], 0, [[WS, HS], [WS - 1, 2]])]
                    for r in rings:
                        # memset lacks an f32r encoding; zero via an fp32
                        # view, then a same-place copy gives the verifier
                        # an f32r-rounding producer.
                        nc.gpsimd.memset(r.bitcast(dt.float32), 0.0)
                        nc.gpsimd.tensor_copy(r, r.bitcast(dt.float32))
                    spi = _ap(sp[:, 0, 0], WS + 1, [[WS, HO], [1, WO]])
                    nc.gpsimd.tensor_scalar(spi, v, tau[:, :], None, Alu.is_gt)
                    spb = sp[:, 0, 0]
                    for c in range(2):
                        ps3 = psum.tile([COUT, CHUNK], dt.float32, tag="mm")
                        for k in range(9):
                            kh, kw = divmod(k, 3)
                            rhs = _ap(spb, kh * WS + kw + c * RC * WS,
                                      [[WS, RC], [1, WO]])
                            nc.tensor.matmul(ps3[:, :], w2[:, k, :], rhs,
                                             start=(k == 0), stop=(k == 8))
                        o2 = off + c * CHUNK
                        nc.scalar.copy(cv2f[:, o2:o2 + CHUNK], ps3[:, :])
                        nc.vector.bn_stats(out=st2[:, 2 * it + c, :], in_=ps3[:, :])

        # ---- AllReduce #2 (bn2 stats) ----
        mv2 = const.tile([COUT, 2], dt.float32)
        nc.vector.bn_aggr(out=mv2[:, :], in_=st2[:, :, :])
        ar2 = const.tile([COUT, 2], dt.float32)
        nc.vector.tensor_scalar_mul(ar2[:, 0:1], mv2[:, 0:1], float(NLOC))
        nc.vector.scalar_tensor_tensor(ar2[:, 1:2], mv2[:, 0:1], float(NLOC),
                                       mv2[:, 0:1], Alu.mult, Alu.mult)
        nc.vector.scalar_tensor_tensor(ar2[:, 1:2], mv2[:, 1:2], float(NLOC),
                                       ar2[:, 1:2], Alu.mult, Alu.add)
        gs2 = gathered_sum(ar2[:, :], 2, "cc2")

        a2, b2 = mk_bn_consts(gs2[:, 0:2], params["bn2_g"], params["bn2_b"], "bn2")
        nb2 = const.tile([COUT, 1], dt.float32)
        nc.vector.tensor_scalar_mul(nb2[:, :], b2[:, :], -1.0)

        # out = 1[a2*cv2 + b2 > s']  ->  tmp = a2*cv2 - s' (DVE), then
        # out_u8 = tmp > -b2 (Pool), one batched uint8 DMA per sample.
        with tc.tile_pool(name="outp", bufs=1) as op, \
             tc.tile_pool(name="tmpp", bufs=4) as tp:
            ob = op.tile([COUT, NLOC], dt.uint8)
            for s in range(BPC):
                for t in range(T):
                    it = s * T + t
                    off = it * NPIX
                    tmp = tp.tile([COUT, NPIX], dt.float32, tag="tm")
                    nc.vector.scalar_tensor_tensor(
                        tmp[:, :], cv2f[:, off:off + NPIX], a2[:, :],
                        scf[:, off:off + NPIX], Alu.mult, Alu.subtract)
                    # compare+uint8 convert on Pool (DVE is the busier engine
                    # here: 16x 877ns stt vs Pool 16x 653ns is_gt)
                    nc.gpsimd.tensor_scalar(ob[:, off:off + NPIX], tmp[:, :],
                                            nb2[:, :], None, Alu.is_gt)
                nc.sync.dma_start(
                    out=out_d.ap()[s, :, :, :, :].rearrange("c t h w -> c (t h w)"),
                    in_=ob[:, s * T * NPIX:(s + 1) * T * NPIX])

    nc.compile()
    return nc


_CACHE = {}


def prep_x(x):
    """Host-side: pad one leading zero row+col, split into bf16 hi/lo.

    Returns (xhh, xlo): xhh [B, 2*CIN, T, HP, WP] holds Xhi duplicated on
    both channel halves; xlo [B, CIN, T, HP, WP] is the bf16 residual.
    """
    import ml_dtypes
    xp = np.zeros(x.shape[:-2] + (HP, WP), np.float32)
    xp[..., 1:, 1:] = x
    xhi = xp.astype(ml_dtypes.bfloat16)
    xlo = (xp - xhi.astype(np.float32)).astype(ml_dtypes.bfloat16)
    xhh = np.concatenate([xhi, xhi], axis=1)
    return np.ascontiguousarray(xhh), np.ascontiguousarray(xlo)


def split_hilo(w):
    """bf16 hi/lo split along a new leading partition block: [2*K, ...]."""
    import ml_dtypes
    hi = w.astype(ml_dtypes.bfloat16)
    lo = (w - hi.astype(np.float32)).astype(ml_dtypes.bfloat16)
    return np.ascontiguousarray(np.concatenate([hi, lo], axis=0))


def kernel(**inputs):
    xhh, xlo = prep_x(np.asarray(inputs["x"], dtype=np.float32))
    w1 = np.ascontiguousarray(inputs["cv1_w"], np.float32).reshape(COUT, CIN, 9)
    w1c = split_hilo(w1.transpose(1, 2, 0))                    # [2i, k, o] bf16
    w2 = np.ascontiguousarray(inputs["cv2_w"], np.float32).reshape(COUT, COUT, 9)
    w2t = np.ascontiguousarray(w2.transpose(1, 2, 0))          # [i, k, o]
    ws = np.ascontiguousarray(inputs["sc_w"], np.float32).reshape(COUT, CIN)
    wsc = split_hilo(ws.transpose(1, 0))                       # [2i, o] bf16
    pars = {p: np.ascontiguousarray(inputs[p], np.float32).reshape(COUT, 1)
            for p in ["bn1_g", "bn1_b", "bn2_g", "bn2_b", "scn_g", "scn_b"]}
    d = float(1.0 / (1.0 + math.exp(-float(np.asarray(inputs["decay"]).ravel()[0]))))

    key = round(d, 12)
    if key not in _CACHE:
        _CACHE[key] = build_nc(d)
    nc = _CACHE[key]

    in_maps = []
    for c in range(NCORES):
        m = {"xhh": xhh[c * BPC:(c + 1) * BPC],
             "xlo": xlo[c * BPC:(c + 1) * BPC],
             "w1c": w1c, "w2t": w2t, "wsc": wsc}
        m.update(pars)
        in_maps.append(m)
    res = run_bass_kernel_spmd(nc, in_maps, core_ids=list(range(NCORES)))
    out = np.concatenate([res.results[c]["out"] for c in range(NCORES)], axis=0)
    return out.astype(np.float32)
